# revision 21
# baseline (speedup 1.0000x reference)
"""Trainium2 Bass kernel: contrastive loss over N=8192, D=512 embeddings.

loss = sum_i[ ln(sum_j exp(2 * ne_i . ne_j)) - 2 * ne_i . nt_i ] / (2N)
with ne = normalize(embeddings, dim=1), nt = normalize(targets, dim=1).

Sharding: row-parallel. Core p owns rows [p*1024, (p+1)*1024). Each core
normalizes its block, transposes it to [D, 1024] (bf16), AllGathers the
transposed normalized blocks (1MB/core), then computes its 1024x8192 slice
of the similarity matrix as a dense bf16 GEMM. exp(2x) + row-sum are fused
on the scalar engine (activation accum_out) reading 4-bank PSUM groups.
Each core emits one partial scalar; the host sums 8 partials / (2N).
"""

import numpy as np

import concourse.bass as bass  # noqa: F401  (registers engines)
import concourse.mybir as mybir
import concourse.tile as tile
from concourse import bacc
from concourse.bass_utils import run_bass_kernel_spmd
from concourse.masks import make_identity

N, D = 8192, 512
NCORES = 8
BLK = N // NCORES        # 1024 rows per core
TILES = BLK // 128       # 8 row tiles of 128 per core
DCH = D // 128           # 4 contraction chunks of 128
PAIRS = NCORES // 2      # j-panel pairs (each pair = 2048 columns = 4 PSUM banks)

FP32 = mybir.dt.float32
BF16 = mybir.dt.bfloat16
OP = mybir.AluOpType
AF = mybir.ActivationFunctionType
AX = mybir.AxisListType


def _body(nc, tc, emb, tgt, partial, no_collective=False, n_ag=2, merge_dma=True):
    with (
        tc.tile_pool(name="io", bufs=1) as io,
        tc.tile_pool(name="persist", bufs=1) as persist,
        tc.tile_pool(name="work", bufs=3) as work,
        tc.tile_pool(name="kxn", bufs=2) as kxnp,
        tc.tile_pool(name="ps", bufs=2, space="PSUM") as psp,
        tc.tile_pool(name="dram", bufs=1, space="DRAM") as dram,
    ):
        NAG = n_ag       # number of AllGathers the transposed block is split into
        HT = TILES // NAG  # tiles per split
        HW_ = HT * 128     # columns per split

        # ---- Normalized-transposed own block, built half by half so the
        #      first AllGather can fire while the second half is still being
        #      normalized. Block row t*128+p -> (p, t).
        ident = persist.tile([128, 128], BF16)
        make_identity(nc, ident)
        emb_sb = io.tile([128, TILES, D], FP32)
        neT = persist.tile([128, DCH, BLK], BF16)
        n2e = persist.tile([128, TILES], FP32)
        inv_e = persist.tile([128, TILES], FP32)
        ne = persist.tile([128, TILES, D], BF16)
        cc_in = [dram.tile([DCH, 128, HW_], BF16, name=f"cc_in{h}")
                 for h in range(NAG)]
        cc_out = [
            dram.tile([NCORES, DCH, 128, HW_], BF16, name=f"cc_out{h}",
                      addr_space="Local" if no_collective else "Shared")
            for h in range(NAG)
        ]
        emb_r = emb.rearrange("(t p) d -> p t d", p=128)

        for h in range(NAG):
            ts_ = range(h * HT, (h + 1) * HT)
            nc.sync.dma_start(emb_sb[:, h * HT : (h + 1) * HT], emb_r[:, h * HT : (h + 1) * HT])
            for t in ts_:
                sq = work.tile([128, D], FP32, tag="sq")
                nc.vector.tensor_tensor(sq, emb_sb[:, t], emb_sb[:, t], OP.mult)
                nc.vector.tensor_reduce(n2e[:, t : t + 1], sq[:], axis=AX.X, op=OP.add)
            # 1/sqrt: sloppy HW sqrt + reciprocal + one Newton step.
            n2h = n2e[:, h * HT : (h + 1) * HT]
            invh = inv_e[:, h * HT : (h + 1) * HT]
            y = work.tile([128, HT], FP32, tag="y")
            nc.scalar.activation(y, n2h, AF.Sqrt)
            nc.vector.reciprocal(y, y)
            t1 = work.tile([128, HT], FP32, tag="t1")
            nc.vector.tensor_tensor(t1, y, y, OP.mult)
            nc.vector.tensor_tensor(t1, t1, n2h, OP.mult)
            nc.vector.tensor_scalar(t1, t1, -0.5, 1.5, OP.mult, OP.add)
            nc.vector.tensor_tensor(invh, y, t1, OP.mult)
            for t in ts_:
                nc.vector.tensor_scalar_mul(ne[:, t], emb_sb[:, t], inv_e[:, t : t + 1])
                for c in range(DCH):
                    pst = psp.tile([128, 128], BF16, tag="ps")
                    nc.tensor.transpose(pst, ne[:, t, c * 128 : (c + 1) * 128], ident)
                    nc.scalar.copy(neT[:, c, t * 128 : (t + 1) * 128], pst)
            nc.sync.dma_start(
                cc_in[h].rearrange("c p w -> p c w"),
                neT[:, :, h * HW_ : (h + 1) * HW_])
            if no_collective:
                for q in range(NCORES):
                    nc.sync.dma_start(cc_out[h][q], cc_in[h][:])
            else:
                nc.gpsimd.collective_compute(
                    "AllGather", OP.bypass,
                    replica_groups=[list(range(NCORES))],
                    ins=[cc_in[h][:].opt()], outs=[cc_out[h][:].opt()])

        # ---- Positive-pair dots (off the critical path; overlaps the GEMM).
        tgt_sb = io.tile([128, TILES, D], FP32)
        nc.sync.dma_start(tgt_sb[:], tgt.rearrange("(t p) d -> p t d", p=128))
        n2t = persist.tile([128, TILES], FP32)
        for t in range(TILES):
            sq2 = work.tile([128, D], FP32, tag="sq")
            nc.vector.tensor_tensor(sq2, tgt_sb[:, t], tgt_sb[:, t], OP.mult)
            nc.vector.tensor_reduce(n2t[:, t : t + 1], sq2[:], axis=AX.X, op=OP.add)
        inv_t = persist.tile([128, TILES], FP32)
        y2 = work.tile([128, TILES], FP32, tag="y2")
        nc.scalar.activation(y2, n2t, AF.Sqrt)
        nc.vector.reciprocal(y2, y2)
        t2 = work.tile([128, TILES], FP32, tag="t2")
        nc.vector.tensor_tensor(t2, y2, y2, OP.mult)
        nc.vector.tensor_tensor(t2, t2, n2t, OP.mult)
        nc.vector.tensor_scalar(t2, t2, -0.5, 1.5, OP.mult, OP.add)
        nc.vector.tensor_tensor(inv_t, y2, t2, OP.mult)
        pos = persist.tile([128, TILES], FP32)
        for t in range(TILES):
            ntt = work.tile([128, D], BF16, tag="ntt")
            nc.vector.tensor_scalar_mul(ntt, tgt_sb[:, t], inv_t[:, t : t + 1])
            pr = work.tile([128, D], FP32, tag="pr")
            nc.vector.tensor_tensor(pr, ne[:, t], ntt[:], OP.mult)
            nc.vector.tensor_reduce(pos[:, t : t + 1], pr[:], axis=AX.X, op=OP.add)

        # ---- Row-block GEMM + fused exp/row-sum.
        # Groups of 2048 j-columns (4 PSUM banks): 4 block-halves from one
        # AllGather. 16 matmuls accumulate the 4 contraction chunks, then one
        # scalar-engine Exp with accum_out emits the partial row sums while
        # the PE fills the other PSUM group. h=0 groups only need the first
        # AllGather, so the GEMM starts while the second is in flight.
        S_parts = persist.tile([128, TILES * PAIRS], FP32)
        n_groups_h = (NCORES * HW_) // 2048
        for h in range(NAG):
            for gq in range(n_groups_h):
                kxn = kxnp.tile([128, DCH, 2048], BF16, tag="kxn")
                q0 = (gq * 2048) // HW_
                nq = 2048 // HW_
                if merge_dma:
                    for c in range(DCH):
                        nc.sync.dma_start(
                            kxn[:, c],
                            cc_out[h][q0 : q0 + nq, c].rearrange("q p w -> p q w"))
                else:
                    for c in range(DCH):
                        for jj in range(4):
                            fc = gq * 2048 + jj * 512
                            q, off = fc // HW_, fc % HW_
                            nc.sync.dma_start(
                                kxn[:, c, jj * 512 : (jj + 1) * 512],
                                cc_out[h][q, c][:, off : off + 512])
                g = h * n_groups_h + gq
                for it in range(TILES):
                    ps = psp.tile([128, 2048], FP32, tag="ps")
                    for c in range(DCH):
                        for jj in range(4):
                            nc.tensor.matmul(
                                ps[:, jj * 512 : (jj + 1) * 512],
                                neT[:, c, it * 128 : (it + 1) * 128],
                                kxn[:, c, jj * 512 : (jj + 1) * 512],
                                start=(c == 0), stop=(c == DCH - 1))
                    nc.scalar.activation(
                        ps, ps, AF.Exp, scale=2.0,
                        accum_out=S_parts[:, it * PAIRS + g : it * PAIRS + g + 1])

        # ---- loss rows: ln(S_i) - 2*pos_i, then reduce the block to a scalar.
        S = persist.tile([128, TILES], FP32)
        nc.vector.tensor_reduce(
            S, S_parts.rearrange("p (t q) -> p t q", q=PAIRS), axis=AX.X, op=OP.add)
        lnS = persist.tile([128, TILES], FP32)
        nc.scalar.activation(lnS, S, AF.Ln)
        lrow = persist.tile([128, TILES], FP32)
        nc.vector.scalar_tensor_tensor(
            out=lrow, in0=pos, scalar=-2.0, in1=lnS, op0=OP.mult, op1=OP.add)

        ones = persist.tile([128, 1], FP32)
        nc.vector.memset(ones, 1.0)
        psf = psp.tile([1, TILES], FP32, tag="ps")
        nc.tensor.matmul(psf, ones, lrow, start=True, stop=True)
        red = persist.tile([1, TILES], FP32)
        nc.scalar.copy(red, psf)
        out_sb = persist.tile([1, 1], FP32)
        nc.vector.tensor_reduce(out_sb, red, axis=AX.X, op=OP.add)
        nc.sync.dma_start(partial, out_sb)


def build_program(no_collective=False, repeat=1, n_ag=2, merge_dma=True):
    nc = bacc.Bacc(
        "TRN2", target_bir_lowering=False, debug=False, num_devices=NCORES)
    emb = nc.dram_tensor("emb_block", [BLK, D], FP32, kind="ExternalInput").ap()
    tgt = nc.dram_tensor("tgt_block", [BLK, D], FP32, kind="ExternalInput").ap()
    partial = nc.dram_tensor("partial", [1, 1], FP32, kind="ExternalOutput").ap()
    with tile.TileContext(nc) as tc:
        for rep in range(repeat):
            if rep:
                tc.strict_bb_all_engine_barrier()
            _body(nc, tc, emb, tgt, partial, no_collective=no_collective,
                  n_ag=n_ag, merge_dma=merge_dma)
    nc.compile()
    return nc


_NC = None


def _get_nc():
    global _NC
    if _NC is None:
        _NC = build_program()
    return _NC


def make_in_maps(emb, tgt):
    return [
        {
            "emb_block": np.ascontiguousarray(emb[p * BLK : (p + 1) * BLK]),
            "tgt_block": np.ascontiguousarray(tgt[p * BLK : (p + 1) * BLK]),
        }
        for p in range(NCORES)
    ]


def run_on_hw(emb, tgt, **kw):
    nc = _get_nc()
    return run_bass_kernel_spmd(nc, make_in_maps(emb, tgt), list(range(NCORES)), **kw)


def finish(partials):
    total = float(np.sum(np.asarray(partials, dtype=np.float64)))
    return np.asarray(total / (2.0 * N), dtype=np.float32)


def kernel(embeddings, targets):
    emb = np.asarray(embeddings, dtype=np.float32)
    tgt = np.asarray(targets, dtype=np.float32)
    res = run_on_hw(emb, tgt)
    partials = [float(r["partial"][0, 0]) for r in res.results]
    return finish(partials)


if __name__ == "__main__":
    rng = np.random.default_rng(0)
    e = rng.standard_normal((N, D), dtype=np.float32)
    t = rng.standard_normal((N, D), dtype=np.float32)
    print("loss:", kernel(e, t))
